# revision 13
# baseline (speedup 1.0000x reference)
"""MoE router kernel (nn_MoELayerWrapper) for 8 TRN2 NeuronCores.

Computes, for hidden_states [B=4, S=4096, D=4096] f32 and w_router [D, E=64] f32:
    router_logits = hidden_states @ w_router            # [B,S,E]
    routing_probs = softmax(router_logits, axis=-1)
    current_load  = routing_probs.sum(axis=(0,1))       # [E] (global over all tokens)
    load_penalty  = current_load / (current_load.mean() + 1e-10)
    adjusted      = router_logits - log(load_penalty + 1e-10)
    expert_idx    = top_k(adjusted, 2).indices          # [B,S,2] int32
returns (adjusted [B,S,E] f32, expert_idx [B,S,2] int32)

Strategy: token (B*S) sharding across 8 cores, w replicated. The matmul runs
as an fp16 hi/lo split (x = h0 + h1, w*1024 = g0 + g1 packed side by side in
the 128-wide stationary operand) so the TensorEngine streams at bf16 rate
while the result is fp32-faithful (~1e-6 abs). Inputs stream in k-quarter
tiles so the PE starts early and never idles long enough to re-throttle.
Logits are transposed to token-major inside the DMA-shadowed main loop;
softmax row-sums come from the scalar engine's accum_out.

The global load reduction runs as two launches: launch A produces token-major
logits + this core's expert-load row; the host gathers the 8 tiny load rows
(the only cross-core traffic, 8x256B); launch B reduces them and applies
penalty + top-2 on device. This avoids collective_compute, which measured
~0.5 ms per AllReduce in this runtime — far more than launch B's ~10 us.

Outputs are written in the on-chip layout and unshuffled on host.
Hardcoded shapes per the problem spec.
"""

import numpy as np

B, S, D, E = 4, 4096, 4096, 64
TOP_K = 2
EPS = 1e-10
N_CORES = 8
T = B * S                 # 16384 tokens
TPC = T // N_CORES        # 2048 tokens per core
NT = 512                  # tokens per compute chunk
NCH = TPC // NT           # 4 chunks
NTT = TPC // 128          # 16 token-major tiles of 128
KT = D // 128             # 32 contraction tiles
KQ = 4                    # k-quarters per chunk
KTQ = KT // KQ            # 8 k-tiles per quarter
WSCALE = 1024.0           # keeps the w-residual split in fp16 normal range

_CACHE = {}


def _mk(name):
    import concourse.mybir as mybir  # noqa: F401
    return mybir


def _build_a():
    import concourse.bacc as bacc
    import concourse.mybir as mybir
    import concourse.tile as tile

    FP32 = mybir.dt.float32
    FP16 = mybir.dt.float16
    ALU = mybir.AluOpType
    ACTF = mybir.ActivationFunctionType

    nc = bacc.Bacc("TRN2", target_bir_lowering=False, debug=False,
                   num_devices=N_CORES, name="moe_a")

    xh0_d = nc.dram_tensor("xh0", [D, TPC], FP16, kind="ExternalInput")
    xh1_d = nc.dram_tensor("xh1", [D, TPC], FP16, kind="ExternalInput")
    # gcat host-relayout: row p holds [KT, 128] for d = k*128 + p (8 KB rows)
    gcat_d = nc.dram_tensor("gcat", [128, KT * 2 * E], FP16, kind="ExternalInput")
    ident_d = nc.dram_tensor("ident", [E, E], FP32, kind="ExternalInput")
    ltm_d = nc.dram_tensor("ltm", [128, NTT * E], FP32, kind="ExternalOutput")
    loadrow_d = nc.dram_tensor("loadrow", [1, E], FP32, kind="ExternalOutput")

    with tile.TileContext(nc) as tc:
        with (
            tc.tile_pool(name="sb", bufs=2) as sb,
            tc.tile_pool(name="ps", bufs=2, space="PSUM") as ps,
        ):
            gcat = sb.tile([128, KT, 2 * E], FP16, tag="gcat")
            ident = sb.tile([E, E], FP32, tag="ident")
            ones128 = sb.tile([128, 1], FP32, tag="ones128")

            xg = xh0_d.ap().rearrange("(k p) t -> p k t", p=128)
            xl = xh1_d.ap().rearrange("(k p) t -> p k t", p=128)

            nc.sync.dma_start(out=gcat, in_=gcat_d.ap())
            nc.vector.memset(ones128, 1.0)

            quarters = {}

            def issue_quarter(c, q):
                ts = slice(c * NT, (c + 1) * NT)
                ks = slice(q * KTQ, (q + 1) * KTQ)
                h0 = sb.tile([128, KTQ, NT], FP16, tag="h0q", bufs=6)
                h1 = sb.tile([128, KTQ, NT], FP16, tag="h1q", bufs=6)
                nc.sync.dma_start(out=h0, in_=xg[:, ks, ts])
                nc.sync.dma_start(out=h1, in_=xl[:, ks, ts])
                quarters[(c, q)] = (h0, h1)

            issue_quarter(0, 0)
            issue_quarter(0, 1)
            nc.sync.dma_start(out=ident, in_=ident_d.ap())

            # token-major logits, all 16 tiles resident
            logits_tm = sb.tile([128, NTT, E], FP32, tag="logits_tm")
            load_ps = ps.tile([1, E], FP32, tag="load_ps", bufs=1)

            pending = [(c, q) for c in range(NCH) for q in range(KQ)]
            issued = 2

            for c in range(NCH):
                acc = ps.tile([128, NT], FP32, tag="acc", bufs=2)
                n_mm = 0
                for q in range(KQ):
                    while issued < len(pending) and issued <= c * KQ + q + 2:
                        issue_quarter(*pending[issued])
                        issued += 1
                    h0, h1 = quarters.pop((c, q))
                    for kk in range(KTQ):
                        k = q * KTQ + kk
                        for mv in (h0, h1):
                            nc.tensor.matmul(acc, lhsT=gcat[:, k, :],
                                             rhs=mv[:, kk, :],
                                             start=(n_mm == 0),
                                             stop=(n_mm == 2 * KT - 1))
                            n_mm += 1

                # logitsT chunk = (acc[0:64] + acc[64:128]) / WSCALE
                half = sb.tile([E, NT], FP32, tag="half")
                nc.vector.tensor_scalar(half, acc[E:128, :], 1.0 / WSCALE, None,
                                        op0=ALU.mult)
                logT = sb.tile([E, NT], FP32, tag="logT")
                nc.vector.scalar_tensor_tensor(
                    out=logT, in0=acc[0:E, :], scalar=1.0 / WSCALE, in1=half,
                    op0=ALU.mult, op1=ALU.add)

                # per 128-token tile: transpose, exp(+rowsum), probs, load
                for j in range(NT // 128):
                    i = c * (NT // 128) + j
                    tp = ps.tile([128, E], FP32, tag="tp", bufs=2)
                    nc.tensor.transpose(tp, logT[:, j * 128:(j + 1) * 128], ident)
                    ltile = logits_tm[:, i, :]
                    nc.vector.tensor_copy(ltile, tp)

                    exp_tm = sb.tile([128, E], FP32, tag="exp_tm")
                    rowsum = sb.tile([128, 1], FP32, tag="rowsum")
                    nc.scalar.activation(exp_tm, ltile, ACTF.Exp,
                                         accum_out=rowsum)
                    rrow = sb.tile([128, 1], FP32, tag="rrow")
                    nc.vector.reciprocal(rrow, rowsum)
                    probs = sb.tile([128, E], FP32, tag="probs")
                    nc.vector.tensor_scalar(probs, exp_tm, rrow, None,
                                            op0=ALU.mult)
                    nc.tensor.matmul(load_ps, lhsT=ones128, rhs=probs,
                                     start=(i == 0), stop=(i == NTT - 1))

            load_loc = sb.tile([1, E], FP32, tag="load_loc")
            nc.vector.tensor_copy(load_loc, load_ps)
            nc.sync.dma_start(out=loadrow_d.ap(), in_=load_loc)
            nc.sync.dma_start(out=ltm_d.ap(), in_=logits_tm)

    nc.compile()
    return nc


def _build_b():
    import concourse.bacc as bacc
    import concourse.mybir as mybir
    import concourse.tile as tile

    FP32 = mybir.dt.float32
    I32 = mybir.dt.int32
    U32 = mybir.dt.uint32
    ALU = mybir.AluOpType
    ACTF = mybir.ActivationFunctionType

    nc = bacc.Bacc("TRN2", target_bir_lowering=False, debug=False,
                   num_devices=N_CORES, name="moe_b")

    ltm_d = nc.dram_tensor("ltm", [128, NTT * E], FP32, kind="ExternalInput")
    loads_d = nc.dram_tensor("loads", [N_CORES, E], FP32, kind="ExternalInput")
    adj_d = nc.dram_tensor("adj", [128, NTT * E], FP32, kind="ExternalOutput")
    idx_d = nc.dram_tensor("idx", [128, NTT * TOP_K], I32, kind="ExternalOutput")

    with tile.TileContext(nc) as tc:
        with (
            tc.tile_pool(name="sb", bufs=2) as sb,
            tc.tile_pool(name="ps", bufs=2, space="PSUM") as ps,
        ):
            logits_tm = sb.tile([128, NTT, E], FP32, tag="logits_tm")
            nc.sync.dma_start(out=logits_tm, in_=ltm_d.ap())
            loads = sb.tile([1, N_CORES, E], FP32, tag="loads")
            nc.sync.dma_start(
                out=loads, in_=loads_d.ap())
            ones_1x128 = sb.tile([1, 128], FP32, tag="ones_1x128")
            nc.vector.memset(ones_1x128, 1.0)

            # global load row: sum over the 8 core rows via a transposed AP view
            load_g = sb.tile([1, E], FP32, tag="load_g")
            loads_T = loads[0:1, :, :].rearrange("a c e -> a e c")
            nc.vector.reduce_sum(load_g, loads_T, axis=mybir.AxisListType.X)

            msum = sb.tile([1, 1], FP32, tag="msum")
            nc.vector.reduce_sum(msum, load_g, axis=mybir.AxisListType.X)
            meps = sb.tile([1, 1], FP32, tag="meps")
            nc.vector.tensor_scalar(meps, msum, 1.0 / E, EPS,
                                    op0=ALU.mult, op1=ALU.add)
            rm = sb.tile([1, 1], FP32, tag="rm")
            nc.vector.reciprocal(rm, meps)
            pen = sb.tile([1, E], FP32, tag="pen")
            nc.vector.tensor_scalar(pen, load_g, rm, EPS,
                                    op0=ALU.mult, op1=ALU.add)
            logpen_row = sb.tile([1, E], FP32, tag="logpen_row")
            nc.scalar.activation(logpen_row, pen, ACTF.Ln)

            lp_ps = ps.tile([128, E], FP32, tag="lp_ps", bufs=1)
            nc.tensor.matmul(lp_ps, lhsT=ones_1x128, rhs=logpen_row,
                             start=True, stop=True)
            logpen_bc = sb.tile([128, E], FP32, tag="logpen_bc")
            nc.vector.tensor_copy(logpen_bc, lp_ps)

            adj_all = sb.tile([128, NTT, E], FP32, tag="adj_all")
            mi8_all = sb.tile([128, NTT, 8], U32, tag="mi8_all")
            try:
                lp_b = logpen_bc.to_broadcast([128, NTT, E])
                nc.vector.tensor_tensor(out=adj_all, in0=logits_tm,
                                        in1=lp_b, op=ALU.subtract)
            except Exception:
                for i in range(NTT):
                    nc.vector.tensor_tensor(out=adj_all[:, i, :],
                                            in0=logits_tm[:, i, :],
                                            in1=logpen_bc, op=ALU.subtract)
            for i in range(NTT):
                mx = sb.tile([128, 8], FP32, tag="mx")
                nc.vector.max(out=mx, in_=adj_all[:, i, :])
                nc.vector.max_index(out=mi8_all[:, i, :], in_max=mx,
                                    in_values=adj_all[:, i, :])
            mi_all = sb.tile([128, NTT, TOP_K], I32, tag="mi_all")
            nc.vector.tensor_copy(mi_all, mi8_all[:, :, 0:TOP_K])

            nc.sync.dma_start(out=adj_d.ap(), in_=adj_all)
            nc.sync.dma_start(out=idx_d.ap(), in_=mi_all)

    nc.compile()
    return nc


def _get_ncs():
    if "a" not in _CACHE:
        _CACHE["a"] = _build_a()
    if "b" not in _CACHE:
        _CACHE["b"] = _build_b()
    return _CACHE["a"], _CACHE["b"]


def _prep_in_maps(hidden_states, w_router):
    X = np.asarray(hidden_states, dtype=np.float32).reshape(T, D)
    W = np.asarray(w_router, dtype=np.float32)
    Ws = W * WSCALE
    g0 = Ws.astype(np.float16)
    g1 = (Ws - g0.astype(np.float32)).astype(np.float16)
    gcat = np.concatenate([g0, g1], axis=1)               # [D, 128]
    gcat = np.ascontiguousarray(
        gcat.reshape(KT, 128, 2 * E).transpose(1, 0, 2).reshape(128, KT * 2 * E))
    ident = np.eye(E, dtype=np.float32)
    in_maps = []
    for c in range(N_CORES):
        shard = X[c * TPC:(c + 1) * TPC, :]               # [TPC, D]
        h0 = shard.astype(np.float16)
        h1 = (shard - h0.astype(np.float32)).astype(np.float16)
        in_maps.append({
            "xh0": np.ascontiguousarray(h0.T),            # [D, TPC]
            "xh1": np.ascontiguousarray(h1.T),
            "gcat": gcat,
            "ident": ident,
        })
    return in_maps


def kernel(hidden_states, w_router):
    from concourse.bass_utils import run_bass_kernel_spmd

    nc_a, nc_b = _get_ncs()
    in_maps = _prep_in_maps(hidden_states, w_router)
    res_a = run_bass_kernel_spmd(nc_a, in_maps, list(range(N_CORES)))

    loads = np.concatenate(
        [res_a.results[c]["loadrow"] for c in range(N_CORES)], axis=0)  # [8, E]
    in_maps_b = [{"ltm": res_a.results[c]["ltm"], "loads": loads}
                 for c in range(N_CORES)]
    res_b = run_bass_kernel_spmd(nc_b, in_maps_b, list(range(N_CORES)))

    adjs, idxs = [], []
    for c in range(N_CORES):
        a = res_b.results[c]["adj"].reshape(128, NTT, E)
        adjs.append(np.ascontiguousarray(a.transpose(1, 0, 2)).reshape(TPC, E))
        ix = res_b.results[c]["idx"].reshape(128, NTT, TOP_K)
        idxs.append(np.ascontiguousarray(ix.transpose(1, 0, 2)).reshape(TPC, TOP_K))
    adj = np.concatenate(adjs, axis=0)
    idx = np.concatenate(idxs, axis=0)
    return (adj.reshape(B, S, E).astype(np.float32),
            idx.reshape(B, S, TOP_K).astype(np.int32))


# revision 16
# speedup vs baseline: 1.0219x; 1.0219x over previous
"""MoE router kernel (nn_MoELayerWrapper) for 8 TRN2 NeuronCores.

Computes, for hidden_states [B=4, S=4096, D=4096] f32 and w_router [D, E=64] f32:
    router_logits = hidden_states @ w_router            # [B,S,E]
    routing_probs = softmax(router_logits, axis=-1)
    current_load  = routing_probs.sum(axis=(0,1))       # [E] (global over all tokens)
    load_penalty  = current_load / (current_load.mean() + 1e-10)
    adjusted      = router_logits - log(load_penalty + 1e-10)
    expert_idx    = top_k(adjusted, 2).indices          # [B,S,2] int32
returns (adjusted [B,S,E] f32, expert_idx [B,S,2] int32)

Strategy: token (B*S) sharding across 8 cores, w replicated. The matmul runs
as an fp16 hi/lo split (x = h0 + h1, w*1024 = g0 + g1 packed side by side in
the 128-wide stationary operand) so the TensorEngine streams at bf16 rate
while the result is fp32-faithful (~1e-6 abs). Inputs stream in k-quarter
tiles so the PE starts early and never idles long enough to re-throttle.
Logits are transposed to token-major inside the DMA-shadowed main loop;
softmax row-sums come from the scalar engine's accum_out.

The global load reduction runs as two launches: launch A produces token-major
logits + this core's expert-load row; the host gathers the 8 tiny load rows
(the only cross-core traffic, 8x256B); launch B reduces them and applies
penalty + top-2 on device. This avoids collective_compute, which measured
~0.5 ms per AllReduce in this runtime — far more than launch B's ~13 us.

Outputs are written in the on-chip layout and unshuffled on host.
Hardcoded shapes per the problem spec.
"""

import numpy as np

B, S, D, E = 4, 4096, 4096, 64
TOP_K = 2
EPS = 1e-10
N_CORES = 8
T = B * S                 # 16384 tokens
TPC = T // N_CORES        # 2048 tokens per core
NT = 512                  # tokens per compute chunk
NCH = TPC // NT           # 4 chunks
NTT = TPC // 128          # 16 token-major tiles of 128
KT = D // 128             # 32 contraction tiles
KQ = 4                    # k-quarters per chunk
KTQ = KT // KQ            # 8 k-tiles per quarter
WSCALE = 1024.0           # keeps the w-residual split in fp16 normal range

_CACHE = {}


def _mk(name):
    import concourse.mybir as mybir  # noqa: F401
    return mybir


def _build_a():
    import concourse.bacc as bacc
    import concourse.mybir as mybir
    import concourse.tile as tile

    FP32 = mybir.dt.float32
    FP16 = mybir.dt.float16
    ALU = mybir.AluOpType
    ACTF = mybir.ActivationFunctionType

    nc = bacc.Bacc("TRN2", target_bir_lowering=False, debug=False,
                   num_devices=N_CORES, name="moe_a")

    # host packs each (chunk, quarter) granule contiguously per partition:
    # [128, NCH, KQ, KTQ, NT] flattened to [128, TPC*KT/128]
    xh0_d = nc.dram_tensor("xh0", [128, NCH * KQ * KTQ * NT], FP16,
                           kind="ExternalInput")
    xh1_d = nc.dram_tensor("xh1", [128, NCH * KQ * KTQ * NT], FP16,
                           kind="ExternalInput")
    # gcat host-relayout: row p holds [KT, 128] for d = k*128 + p (8 KB rows)
    gcat_d = nc.dram_tensor("gcat", [128, KT * 2 * E], FP16, kind="ExternalInput")
    ident_d = nc.dram_tensor("ident", [E, E], FP32, kind="ExternalInput")
    ltm_d = nc.dram_tensor("ltm", [128, NTT * E], FP32, kind="ExternalOutput")
    loadrow_d = nc.dram_tensor("loadrow", [1, E], FP32, kind="ExternalOutput")

    with tile.TileContext(nc) as tc:
        with (
            tc.tile_pool(name="sb", bufs=2) as sb,
            tc.tile_pool(name="ps", bufs=2, space="PSUM") as ps,
        ):
            gcat = sb.tile([128, KT, 2 * E], FP16, tag="gcat")
            ident = sb.tile([E, E], FP32, tag="ident")
            ones128 = sb.tile([128, 1], FP32, tag="ones128")

            QELEM = KTQ * NT
            xg = xh0_d.ap()
            xl = xh1_d.ap()

            nc.vector.memset(ones128, 1.0)

            quarters = {}

            def issue_quarter(c, q):
                s = slice((c * KQ + q) * QELEM, (c * KQ + q + 1) * QELEM)
                h0 = sb.tile([128, KTQ, NT], FP16, tag="h0q", bufs=6)
                h1 = sb.tile([128, KTQ, NT], FP16, tag="h1q", bufs=6)
                nc.sync.dma_start(out=h0, in_=xg[:, s])
                nc.sync.dma_start(out=h1, in_=xl[:, s])
                quarters[(c, q)] = (h0, h1)

            # interleave first-quarter data with gcat k-slices so the PE
            # can start as early as possible
            GQE = KTQ * 2 * E
            issue_quarter(0, 0)
            nc.sync.dma_start(out=gcat[:, 0:KTQ, :],
                              in_=gcat_d.ap()[:, 0:GQE])
            issue_quarter(0, 1)
            for gq in range(1, KQ):
                nc.sync.dma_start(out=gcat[:, gq * KTQ:(gq + 1) * KTQ, :],
                                  in_=gcat_d.ap()[:, gq * GQE:(gq + 1) * GQE])
            nc.sync.dma_start(out=ident, in_=ident_d.ap())

            # token-major logits, all 16 tiles resident
            logits_tm = sb.tile([128, NTT, E], FP32, tag="logits_tm")
            load_ps = ps.tile([1, E], FP32, tag="load_ps", bufs=1)

            pending = [(c, q) for c in range(NCH) for q in range(KQ)]
            issued = 2

            for c in range(NCH):
                acc = ps.tile([128, NT], FP32, tag="acc", bufs=2)
                n_mm = 0
                for q in range(KQ):
                    while issued < len(pending) and issued <= c * KQ + q + 2:
                        issue_quarter(*pending[issued])
                        issued += 1
                    h0, h1 = quarters.pop((c, q))
                    for kk in range(KTQ):
                        k = q * KTQ + kk
                        for mv in (h0, h1):
                            nc.tensor.matmul(acc, lhsT=gcat[:, k, :],
                                             rhs=mv[:, kk, :],
                                             start=(n_mm == 0),
                                             stop=(n_mm == 2 * KT - 1))
                            n_mm += 1

                # logitsT chunk = (acc[0:64] + acc[64:128]) / WSCALE
                half = sb.tile([E, NT], FP32, tag="half")
                nc.vector.tensor_scalar(half, acc[E:128, :], 1.0 / WSCALE, None,
                                        op0=ALU.mult)
                logT = sb.tile([E, NT], FP32, tag="logT")
                nc.vector.scalar_tensor_tensor(
                    out=logT, in0=acc[0:E, :], scalar=1.0 / WSCALE, in1=half,
                    op0=ALU.mult, op1=ALU.add)

                # per 128-token tile: transpose, exp(+rowsum), probs, load
                for j in range(NT // 128):
                    i = c * (NT // 128) + j
                    tp = ps.tile([128, E], FP32, tag="tp", bufs=2)
                    nc.tensor.transpose(tp, logT[:, j * 128:(j + 1) * 128], ident)
                    ltile = logits_tm[:, i, :]
                    nc.vector.tensor_copy(ltile, tp)

                    exp_tm = sb.tile([128, E], FP32, tag="exp_tm")
                    rowsum = sb.tile([128, 1], FP32, tag="rowsum")
                    nc.scalar.activation(exp_tm, ltile, ACTF.Exp,
                                         accum_out=rowsum)
                    rrow = sb.tile([128, 1], FP32, tag="rrow")
                    nc.vector.reciprocal(rrow, rowsum)
                    probs = sb.tile([128, E], FP32, tag="probs")
                    nc.vector.tensor_scalar(probs, exp_tm, rrow, None,
                                            op0=ALU.mult)
                    nc.tensor.matmul(load_ps, lhsT=ones128, rhs=probs,
                                     start=(i == 0), stop=(i == NTT - 1))

            nc.sync.dma_start(out=ltm_d.ap(), in_=logits_tm)
            load_loc = sb.tile([1, E], FP32, tag="load_loc")
            nc.vector.tensor_copy(load_loc, load_ps)
            nc.sync.dma_start(out=loadrow_d.ap(), in_=load_loc)

    nc.compile()
    return nc


def _build_b():
    import concourse.bacc as bacc
    import concourse.mybir as mybir
    import concourse.tile as tile

    FP32 = mybir.dt.float32
    I32 = mybir.dt.int32
    U32 = mybir.dt.uint32
    ALU = mybir.AluOpType
    ACTF = mybir.ActivationFunctionType

    nc = bacc.Bacc("TRN2", target_bir_lowering=False, debug=False,
                   num_devices=N_CORES, name="moe_b")

    ltm_d = nc.dram_tensor("ltm", [128, NTT * E], FP32, kind="ExternalInput")
    loads_d = nc.dram_tensor("loads", [N_CORES, E], FP32, kind="ExternalInput")
    adj_d = nc.dram_tensor("adj", [128, NTT * E], FP32, kind="ExternalOutput")
    idx_d = nc.dram_tensor("idx", [128, NTT * TOP_K], I32, kind="ExternalOutput")

    with tile.TileContext(nc) as tc:
        with (
            tc.tile_pool(name="sb", bufs=2) as sb,
            tc.tile_pool(name="ps", bufs=2, space="PSUM") as ps,
        ):
            loads = sb.tile([1, N_CORES, E], FP32, tag="loads")
            nc.sync.dma_start(
                out=loads, in_=loads_d.ap())
            logits_tm = sb.tile([128, NTT, E], FP32, tag="logits_tm")
            QS = NTT // 4
            for s in range(4):
                nc.sync.dma_start(
                    out=logits_tm[:, s * QS:(s + 1) * QS, :],
                    in_=ltm_d.ap()[:, s * QS * E:(s + 1) * QS * E])
            ones_1x128 = sb.tile([1, 128], FP32, tag="ones_1x128")
            nc.vector.memset(ones_1x128, 1.0)

            # global load row: sum over the 8 core rows via a transposed AP view
            load_g = sb.tile([1, E], FP32, tag="load_g")
            loads_T = loads[0:1, :, :].rearrange("a c e -> a e c")
            nc.vector.reduce_sum(load_g, loads_T, axis=mybir.AxisListType.X)

            msum = sb.tile([1, 1], FP32, tag="msum")
            nc.vector.reduce_sum(msum, load_g, axis=mybir.AxisListType.X)
            meps = sb.tile([1, 1], FP32, tag="meps")
            nc.vector.tensor_scalar(meps, msum, 1.0 / E, EPS,
                                    op0=ALU.mult, op1=ALU.add)
            rm = sb.tile([1, 1], FP32, tag="rm")
            nc.vector.reciprocal(rm, meps)
            pen = sb.tile([1, E], FP32, tag="pen")
            nc.vector.tensor_scalar(pen, load_g, rm, EPS,
                                    op0=ALU.mult, op1=ALU.add)
            logpen_row = sb.tile([1, E], FP32, tag="logpen_row")
            nc.scalar.activation(logpen_row, pen, ACTF.Ln)

            lp_ps = ps.tile([128, E], FP32, tag="lp_ps", bufs=1)
            nc.tensor.matmul(lp_ps, lhsT=ones_1x128, rhs=logpen_row,
                             start=True, stop=True)
            logpen_bc = sb.tile([128, E], FP32, tag="logpen_bc")
            nc.vector.tensor_copy(logpen_bc, lp_ps)

            adj_all = sb.tile([128, NTT, E], FP32, tag="adj_all")
            mi8_all = sb.tile([128, NTT, 8], U32, tag="mi8_all")
            # subtract on GpSimd so it overlaps the DVE max/max_index chain
            for i in range(NTT):
                nc.gpsimd.tensor_tensor(out=adj_all[:, i, :],
                                        in0=logits_tm[:, i, :],
                                        in1=logpen_bc, op=ALU.subtract)
            for i in range(NTT):
                mx = sb.tile([128, 8], FP32, tag="mx")
                nc.vector.max(out=mx, in_=adj_all[:, i, :])
                nc.vector.max_index(out=mi8_all[:, i, :], in_max=mx,
                                    in_values=adj_all[:, i, :])
            mi_all = sb.tile([128, NTT, TOP_K], I32, tag="mi_all")
            nc.vector.tensor_copy(mi_all, mi8_all[:, :, 0:TOP_K])

            nc.sync.dma_start(out=adj_d.ap(), in_=adj_all)
            nc.sync.dma_start(out=idx_d.ap(), in_=mi_all)

    nc.compile()
    return nc


def _get_ncs():
    if "a" not in _CACHE:
        _CACHE["a"] = _build_a()
    if "b" not in _CACHE:
        _CACHE["b"] = _build_b()
    return _CACHE["a"], _CACHE["b"]


def _prep_in_maps(hidden_states, w_router):
    X = np.asarray(hidden_states, dtype=np.float32).reshape(T, D)
    W = np.asarray(w_router, dtype=np.float32)
    Ws = W * WSCALE
    g0 = Ws.astype(np.float16)
    g1 = (Ws - g0.astype(np.float32)).astype(np.float16)
    gcat = np.concatenate([g0, g1], axis=1)               # [D, 128]
    gcat = np.ascontiguousarray(
        gcat.reshape(KT, 128, 2 * E).transpose(1, 0, 2).reshape(128, KT * 2 * E))
    ident = np.eye(E, dtype=np.float32)
    def pack(hT):
        # [D, TPC] -> [128, NCH*KQ*KTQ*NT] with (c, q, kk, t) contiguous per
        # partition p, where d = (q*KTQ + kk)*128 + p
        v = hT.reshape(KT, 128, NCH, NT)            # [k, p, c, t]
        v = v.reshape(KQ, KTQ, 128, NCH, NT)        # [q, kk, p, c, t]
        v = v.transpose(2, 3, 0, 1, 4)              # [p, c, q, kk, t]
        return np.ascontiguousarray(v.reshape(128, NCH * KQ * KTQ * NT))

    in_maps = []
    for c in range(N_CORES):
        shard = X[c * TPC:(c + 1) * TPC, :]               # [TPC, D]
        h0 = shard.astype(np.float16)
        h1 = (shard - h0.astype(np.float32)).astype(np.float16)
        in_maps.append({
            "xh0": pack(np.ascontiguousarray(h0.T)),
            "xh1": pack(np.ascontiguousarray(h1.T)),
            "gcat": gcat,
            "ident": ident,
        })
    return in_maps


def kernel(hidden_states, w_router):
    from concourse.bass_utils import run_bass_kernel_spmd

    nc_a, nc_b = _get_ncs()
    in_maps = _prep_in_maps(hidden_states, w_router)
    res_a = run_bass_kernel_spmd(nc_a, in_maps, list(range(N_CORES)))

    loads = np.concatenate(
        [res_a.results[c]["loadrow"] for c in range(N_CORES)], axis=0)  # [8, E]
    in_maps_b = [{"ltm": res_a.results[c]["ltm"], "loads": loads}
                 for c in range(N_CORES)]
    res_b = run_bass_kernel_spmd(nc_b, in_maps_b, list(range(N_CORES)))

    adjs, idxs = [], []
    for c in range(N_CORES):
        a = res_b.results[c]["adj"].reshape(128, NTT, E)
        adjs.append(np.ascontiguousarray(a.transpose(1, 0, 2)).reshape(TPC, E))
        ix = res_b.results[c]["idx"].reshape(128, NTT, TOP_K)
        idxs.append(np.ascontiguousarray(ix.transpose(1, 0, 2)).reshape(TPC, TOP_K))
    adj = np.concatenate(adjs, axis=0)
    idx = np.concatenate(idxs, axis=0)
    return (adj.reshape(B, S, E).astype(np.float32),
            idx.reshape(B, S, TOP_K).astype(np.int32))


# revision 17
# speedup vs baseline: 1.0328x; 1.0107x over previous
"""MoE router kernel (nn_MoELayerWrapper) for 8 TRN2 NeuronCores.

Computes, for hidden_states [B=4, S=4096, D=4096] f32 and w_router [D, E=64] f32:
    router_logits = hidden_states @ w_router            # [B,S,E]
    routing_probs = softmax(router_logits, axis=-1)
    current_load  = routing_probs.sum(axis=(0,1))       # [E] (global over all tokens)
    load_penalty  = current_load / (current_load.mean() + 1e-10)
    adjusted      = router_logits - log(load_penalty + 1e-10)
    expert_idx    = top_k(adjusted, 2).indices          # [B,S,2] int32
returns (adjusted [B,S,E] f32, expert_idx [B,S,2] int32)

Strategy: token (B*S) sharding across 8 cores, w replicated. The matmul runs
as an fp16 hi/lo split (x = h0 + h1, w*1024 = g0 + g1 packed side by side in
the 128-wide stationary operand) so the TensorEngine streams at bf16 rate
while the result is fp32-faithful (~1e-6 abs). Inputs stream in k-quarter
tiles so the PE starts early and never idles long enough to re-throttle.
Logits are transposed to token-major inside the DMA-shadowed main loop;
softmax row-sums come from the scalar engine's accum_out.

The global load reduction runs as two launches: launch A produces token-major
logits + this core's expert-load row; the host gathers the 8 tiny load rows
(the only cross-core traffic, 8x256B); launch B reduces them and applies
penalty + top-2 on device. This avoids collective_compute, which measured
~0.5 ms per AllReduce in this runtime — far more than launch B's ~13 us.

Outputs are written in the on-chip layout and unshuffled on host.
Hardcoded shapes per the problem spec.
"""

import numpy as np

B, S, D, E = 4, 4096, 4096, 64
TOP_K = 2
EPS = 1e-10
N_CORES = 8
T = B * S                 # 16384 tokens
TPC = T // N_CORES        # 2048 tokens per core
NT = 256                  # tokens per compute chunk
NCH = TPC // NT           # 4 chunks
NTT = TPC // 128          # 16 token-major tiles of 128
KT = D // 128             # 32 contraction tiles
KQ = 2                    # k-granules per chunk (1 MB each)
KTQ = KT // KQ            # 8 k-tiles per quarter
WSCALE = 1024.0           # keeps the w-residual split in fp16 normal range

_CACHE = {}


def _mk(name):
    import concourse.mybir as mybir  # noqa: F401
    return mybir


def _build_a():
    import concourse.bacc as bacc
    import concourse.mybir as mybir
    import concourse.tile as tile

    FP32 = mybir.dt.float32
    FP16 = mybir.dt.float16
    ALU = mybir.AluOpType
    ACTF = mybir.ActivationFunctionType

    nc = bacc.Bacc("TRN2", target_bir_lowering=False, debug=False,
                   num_devices=N_CORES, name="moe_a")

    # host packs each (chunk, quarter) granule contiguously per partition:
    # [128, NCH, KQ, KTQ, NT] flattened to [128, TPC*KT/128]
    xh0_d = nc.dram_tensor("xh0", [128, NCH * KQ * KTQ * NT], FP16,
                           kind="ExternalInput")
    xh1_d = nc.dram_tensor("xh1", [128, NCH * KQ * KTQ * NT], FP16,
                           kind="ExternalInput")
    # gcat host-relayout: row p holds [KT, 128] for d = k*128 + p (8 KB rows)
    gcat_d = nc.dram_tensor("gcat", [128, KT * 2 * E], FP16, kind="ExternalInput")
    ident_d = nc.dram_tensor("ident", [E, E], FP32, kind="ExternalInput")
    ltm_d = nc.dram_tensor("ltm", [128, NTT * E], FP32, kind="ExternalOutput")
    loadrow_d = nc.dram_tensor("loadrow", [1, E], FP32, kind="ExternalOutput")

    with tile.TileContext(nc) as tc:
        with (
            tc.tile_pool(name="sb", bufs=2) as sb,
            tc.tile_pool(name="ps", bufs=2, space="PSUM") as ps,
        ):
            gcat = sb.tile([128, KT, 2 * E], FP16, tag="gcat")
            ident = sb.tile([E, E], FP32, tag="ident")
            ones128 = sb.tile([128, 1], FP32, tag="ones128")

            QELEM = KTQ * NT
            xg = xh0_d.ap()
            xl = xh1_d.ap()

            nc.vector.memset(ones128, 1.0)

            quarters = {}

            def issue_quarter(c, q):
                s = slice((c * KQ + q) * QELEM, (c * KQ + q + 1) * QELEM)
                h0 = sb.tile([128, KTQ, NT], FP16, tag="h0q", bufs=6)
                h1 = sb.tile([128, KTQ, NT], FP16, tag="h1q", bufs=6)
                nc.sync.dma_start(out=h0, in_=xg[:, s])
                nc.sync.dma_start(out=h1, in_=xl[:, s])
                quarters[(c, q)] = (h0, h1)

            # interleave first-quarter data with gcat k-slices so the PE
            # can start as early as possible
            GQE = KTQ * 2 * E
            issue_quarter(0, 0)
            nc.sync.dma_start(out=gcat[:, 0:KTQ, :],
                              in_=gcat_d.ap()[:, 0:GQE])
            issue_quarter(0, 1)
            for gq in range(1, KQ):
                nc.sync.dma_start(out=gcat[:, gq * KTQ:(gq + 1) * KTQ, :],
                                  in_=gcat_d.ap()[:, gq * GQE:(gq + 1) * GQE])
            nc.sync.dma_start(out=ident, in_=ident_d.ap())

            # token-major logits, all 16 tiles resident
            logits_tm = sb.tile([128, NTT, E], FP32, tag="logits_tm")
            load_ps = ps.tile([1, E], FP32, tag="load_ps", bufs=1)

            pending = [(c, q) for c in range(NCH) for q in range(KQ)]
            issued = 2

            for c in range(NCH):
                acc = ps.tile([128, NT], FP32, tag="acc", bufs=2)
                n_mm = 0
                for q in range(KQ):
                    while issued < len(pending) and issued <= c * KQ + q + 2:
                        issue_quarter(*pending[issued])
                        issued += 1
                    h0, h1 = quarters.pop((c, q))
                    for kk in range(KTQ):
                        k = q * KTQ + kk
                        for mv in (h0, h1):
                            nc.tensor.matmul(acc, lhsT=gcat[:, k, :],
                                             rhs=mv[:, kk, :],
                                             start=(n_mm == 0),
                                             stop=(n_mm == 2 * KT - 1))
                            n_mm += 1

                # logitsT chunk = (acc[0:64] + acc[64:128]) / WSCALE
                half = sb.tile([E, NT], FP32, tag="half")
                nc.vector.tensor_scalar(half, acc[E:128, :], 1.0 / WSCALE, None,
                                        op0=ALU.mult)
                logT = sb.tile([E, NT], FP32, tag="logT")
                nc.vector.scalar_tensor_tensor(
                    out=logT, in0=acc[0:E, :], scalar=1.0 / WSCALE, in1=half,
                    op0=ALU.mult, op1=ALU.add)

                # per 128-token tile: transpose, exp(+rowsum), probs, load
                for j in range(NT // 128):
                    i = c * (NT // 128) + j
                    tp = ps.tile([128, E], FP32, tag="tp", bufs=2)
                    nc.tensor.transpose(tp, logT[:, j * 128:(j + 1) * 128], ident)
                    ltile = logits_tm[:, i, :]
                    nc.vector.tensor_copy(ltile, tp)

                    exp_tm = sb.tile([128, E], FP32, tag="exp_tm")
                    rowsum = sb.tile([128, 1], FP32, tag="rowsum")
                    nc.scalar.activation(exp_tm, ltile, ACTF.Exp,
                                         accum_out=rowsum)
                    rrow = sb.tile([128, 1], FP32, tag="rrow")
                    nc.vector.reciprocal(rrow, rowsum)
                    probs = sb.tile([128, E], FP32, tag="probs")
                    nc.vector.tensor_scalar(probs, exp_tm, rrow, None,
                                            op0=ALU.mult)
                    nc.tensor.matmul(load_ps, lhsT=ones128, rhs=probs,
                                     start=(i == 0), stop=(i == NTT - 1))

            nc.sync.dma_start(out=ltm_d.ap(), in_=logits_tm)
            load_loc = sb.tile([1, E], FP32, tag="load_loc")
            nc.vector.tensor_copy(load_loc, load_ps)
            nc.sync.dma_start(out=loadrow_d.ap(), in_=load_loc)

    nc.compile()
    return nc


def _build_b():
    import concourse.bacc as bacc
    import concourse.mybir as mybir
    import concourse.tile as tile

    FP32 = mybir.dt.float32
    I32 = mybir.dt.int32
    U32 = mybir.dt.uint32
    ALU = mybir.AluOpType
    ACTF = mybir.ActivationFunctionType

    nc = bacc.Bacc("TRN2", target_bir_lowering=False, debug=False,
                   num_devices=N_CORES, name="moe_b")

    ltm_d = nc.dram_tensor("ltm", [128, NTT * E], FP32, kind="ExternalInput")
    loads_d = nc.dram_tensor("loads", [N_CORES, E], FP32, kind="ExternalInput")
    adj_d = nc.dram_tensor("adj", [128, NTT * E], FP32, kind="ExternalOutput")
    idx_d = nc.dram_tensor("idx", [128, NTT * TOP_K], I32, kind="ExternalOutput")

    with tile.TileContext(nc) as tc:
        with (
            tc.tile_pool(name="sb", bufs=2) as sb,
            tc.tile_pool(name="ps", bufs=2, space="PSUM") as ps,
        ):
            loads = sb.tile([1, N_CORES, E], FP32, tag="loads")
            nc.sync.dma_start(
                out=loads, in_=loads_d.ap())
            logits_tm = sb.tile([128, NTT, E], FP32, tag="logits_tm")
            QS = NTT // 4
            for s in range(4):
                nc.sync.dma_start(
                    out=logits_tm[:, s * QS:(s + 1) * QS, :],
                    in_=ltm_d.ap()[:, s * QS * E:(s + 1) * QS * E])
            ones_1x128 = sb.tile([1, 128], FP32, tag="ones_1x128")
            nc.vector.memset(ones_1x128, 1.0)

            # global load row: sum over the 8 core rows via a transposed AP view
            load_g = sb.tile([1, E], FP32, tag="load_g")
            loads_T = loads[0:1, :, :].rearrange("a c e -> a e c")
            nc.vector.reduce_sum(load_g, loads_T, axis=mybir.AxisListType.X)

            msum = sb.tile([1, 1], FP32, tag="msum")
            nc.vector.reduce_sum(msum, load_g, axis=mybir.AxisListType.X)
            meps = sb.tile([1, 1], FP32, tag="meps")
            nc.vector.tensor_scalar(meps, msum, 1.0 / E, EPS,
                                    op0=ALU.mult, op1=ALU.add)
            rm = sb.tile([1, 1], FP32, tag="rm")
            nc.vector.reciprocal(rm, meps)
            pen = sb.tile([1, E], FP32, tag="pen")
            nc.vector.tensor_scalar(pen, load_g, rm, EPS,
                                    op0=ALU.mult, op1=ALU.add)
            logpen_row = sb.tile([1, E], FP32, tag="logpen_row")
            nc.scalar.activation(logpen_row, pen, ACTF.Ln)

            lp_ps = ps.tile([128, E], FP32, tag="lp_ps", bufs=1)
            nc.tensor.matmul(lp_ps, lhsT=ones_1x128, rhs=logpen_row,
                             start=True, stop=True)
            logpen_bc = sb.tile([128, E], FP32, tag="logpen_bc")
            nc.vector.tensor_copy(logpen_bc, lp_ps)

            adj_all = sb.tile([128, NTT, E], FP32, tag="adj_all")
            mi8_all = sb.tile([128, NTT, 8], U32, tag="mi8_all")
            # subtract on GpSimd so it overlaps the DVE max/max_index chain
            for i in range(NTT):
                nc.gpsimd.tensor_tensor(out=adj_all[:, i, :],
                                        in0=logits_tm[:, i, :],
                                        in1=logpen_bc, op=ALU.subtract)
            for i in range(NTT):
                mx = sb.tile([128, 8], FP32, tag="mx")
                nc.vector.max(out=mx, in_=adj_all[:, i, :])
                nc.vector.max_index(out=mi8_all[:, i, :], in_max=mx,
                                    in_values=adj_all[:, i, :])
            mi_all = sb.tile([128, NTT, TOP_K], I32, tag="mi_all")
            nc.vector.tensor_copy(mi_all, mi8_all[:, :, 0:TOP_K])

            nc.sync.dma_start(out=adj_d.ap(), in_=adj_all)
            nc.sync.dma_start(out=idx_d.ap(), in_=mi_all)

    nc.compile()
    return nc


def _get_ncs():
    if "a" not in _CACHE:
        _CACHE["a"] = _build_a()
    if "b" not in _CACHE:
        _CACHE["b"] = _build_b()
    return _CACHE["a"], _CACHE["b"]


def _prep_in_maps(hidden_states, w_router):
    X = np.asarray(hidden_states, dtype=np.float32).reshape(T, D)
    W = np.asarray(w_router, dtype=np.float32)
    Ws = W * WSCALE
    g0 = Ws.astype(np.float16)
    g1 = (Ws - g0.astype(np.float32)).astype(np.float16)
    gcat = np.concatenate([g0, g1], axis=1)               # [D, 128]
    gcat = np.ascontiguousarray(
        gcat.reshape(KT, 128, 2 * E).transpose(1, 0, 2).reshape(128, KT * 2 * E))
    ident = np.eye(E, dtype=np.float32)
    def pack(hT):
        # [D, TPC] -> [128, NCH*KQ*KTQ*NT] with (c, q, kk, t) contiguous per
        # partition p, where d = (q*KTQ + kk)*128 + p
        v = hT.reshape(KT, 128, NCH, NT)            # [k, p, c, t]
        v = v.reshape(KQ, KTQ, 128, NCH, NT)        # [q, kk, p, c, t]
        v = v.transpose(2, 3, 0, 1, 4)              # [p, c, q, kk, t]
        return np.ascontiguousarray(v.reshape(128, NCH * KQ * KTQ * NT))

    in_maps = []
    for c in range(N_CORES):
        shard = X[c * TPC:(c + 1) * TPC, :]               # [TPC, D]
        h0 = shard.astype(np.float16)
        h1 = (shard - h0.astype(np.float32)).astype(np.float16)
        in_maps.append({
            "xh0": pack(np.ascontiguousarray(h0.T)),
            "xh1": pack(np.ascontiguousarray(h1.T)),
            "gcat": gcat,
            "ident": ident,
        })
    return in_maps


def kernel(hidden_states, w_router):
    from concourse.bass_utils import run_bass_kernel_spmd

    nc_a, nc_b = _get_ncs()
    in_maps = _prep_in_maps(hidden_states, w_router)
    res_a = run_bass_kernel_spmd(nc_a, in_maps, list(range(N_CORES)))

    loads = np.concatenate(
        [res_a.results[c]["loadrow"] for c in range(N_CORES)], axis=0)  # [8, E]
    in_maps_b = [{"ltm": res_a.results[c]["ltm"], "loads": loads}
                 for c in range(N_CORES)]
    res_b = run_bass_kernel_spmd(nc_b, in_maps_b, list(range(N_CORES)))

    adjs, idxs = [], []
    for c in range(N_CORES):
        a = res_b.results[c]["adj"].reshape(128, NTT, E)
        adjs.append(np.ascontiguousarray(a.transpose(1, 0, 2)).reshape(TPC, E))
        ix = res_b.results[c]["idx"].reshape(128, NTT, TOP_K)
        idxs.append(np.ascontiguousarray(ix.transpose(1, 0, 2)).reshape(TPC, TOP_K))
    adj = np.concatenate(adjs, axis=0)
    idx = np.concatenate(idxs, axis=0)
    return (adj.reshape(B, S, E).astype(np.float32),
            idx.reshape(B, S, TOP_K).astype(np.int32))
